# revision 8
# baseline (speedup 1.0000x reference)
"""Multi-head attention Trainium2 kernel (B=2, S=2048, D=1024, H=16, DH=64).

Sharding: 8 cores; core c handles batch b=c//4 and 4 heads h0=4*(c%4)..h0+4.
Returns (attention (B,S,D) f32, attention_weights (B,H,S,S) f32) like the
reference nn.Module.

Host side pre-transposes activations/weights so every on-device matmul has its
contraction dim on partitions; all matmuls run in float32r (single-pass fp32,
~1.5e-4 rel err). Host also folds bv/bo into the final output (softmax rows
sum to 1, so P @ (V + 1 bv^T) = P@V + 1 bv^T).
"""

import sys

for _p in ("/opt/trn_rl_repo",):
    if _p not in sys.path:
        sys.path.insert(0, _p)

import numpy as np

import concourse.bass as bass
import concourse.mybir as mybir
from concourse import bacc, tile
from concourse.bass_utils import run_bass_kernel_spmd
from concourse.masks import make_identity

F32 = mybir.dt.float32
F32R = mybir.dt.float32r
AF = mybir.ActivationFunctionType
ALU = mybir.AluOpType

B, S, D, H = 2, 2048, 1024, 16
DH = D // H          # 64
HPC = 4              # heads per core
NCORES = 8
PAIRS = HPC // 2     # head pairs per core (partition-stacked 2x64)
KC = D // 128        # contraction chunks for projections (8)
SQC = S // 128       # sq chunks (16)
SKC = S // 128       # sk chunks (16)
NBLK = S // 512      # 512-wide blocks of S (4)
IBLK = S // 256      # 256-row sq blocks (8)
LG = 1024            # logits psum chunk width (2 banks)


def build_kernel(num_devices: int = NCORES):
    nc = bacc.Bacc("TRN2", target_bir_lowering=False, debug=False,
                   num_devices=num_devices)

    # ---- DRAM I/O (per-core shapes) ----
    xqT = nc.dram_tensor("xqT", [D, S], F32R, kind="ExternalInput")
    xkT = nc.dram_tensor("xkT", [D, S], F32R, kind="ExternalInput")
    xvT = nc.dram_tensor("xvT", [D, S], F32R, kind="ExternalInput")
    keep = nc.dram_tensor("keep", [S, S], F32, kind="ExternalInput")
    wqT = nc.dram_tensor("wqT", [D, HPC * DH], F32R, kind="ExternalInput")
    wkT = nc.dram_tensor("wkT", [D, HPC * DH], F32R, kind="ExternalInput")
    wvT = nc.dram_tensor("wvT", [D, HPC * DH], F32R, kind="ExternalInput")
    woT = nc.dram_tensor("woT", [HPC * DH, D], F32R, kind="ExternalInput")
    bqT = nc.dram_tensor("bqT", [128, PAIRS], F32, kind="ExternalInput")
    bkT = nc.dram_tensor("bkT", [128, PAIRS], F32, kind="ExternalInput")
    wout = nc.dram_tensor("wout", [HPC, S, S], F32, kind="ExternalOutput")
    pout = nc.dram_tensor("pout", [S, D], F32, kind="ExternalOutput")

    xqT_r = xqT.rearrange("(k p) s -> k p s", p=128)
    xkT_r = xkT.rearrange("(k p) s -> k p s", p=128)
    xvT_r = xvT.rearrange("(k p) s -> k p s", p=128)
    wqT_r = wqT.rearrange("(k p) m -> k p m", p=128)
    wkT_r = wkT.rearrange("(k p) m -> k p m", p=128)
    wvT_r = wvT.rearrange("(k p) m -> k p m", p=128)
    woT_r = woT.rearrange("(k p) m -> k p m", p=128)
    keep_r = keep.rearrange("(c p) s -> c p s", p=128)

    with tile.TileContext(nc) as tc:
        import contextlib
        ctx = contextlib.ExitStack()
        with ctx:
            persist = ctx.enter_context(tc.tile_pool(name="persist", bufs=1))

            # persistent SBUF tensors
            qt_s = persist.tile([128, PAIRS, S], F32R, tag="qt")
            kt_s = persist.tile([128, PAIRS, S], F32R, tag="kt")
            v_s = persist.tile([128, SKC, HPC * DH], F32R, tag="v")
            at_s = persist.tile([128, PAIRS, S], F32R, tag="at")   # attnT
            wq_s = persist.tile([128, KC, HPC * DH], F32R, tag="wq")
            wk_s = persist.tile([128, KC, HPC * DH], F32R, tag="wk")
            wv_s = persist.tile([128, KC, HPC * DH], F32R, tag="wv")
            wo_s = persist.tile([128, PAIRS, D], F32R, tag="wo")
            bq_s = persist.tile([128, PAIRS], F32, tag="bq")
            bk_s = persist.tile([128, PAIRS], F32, tag="bk")
            ident = persist.tile([128, 128], F32, tag="ident")

            nc.sync.dma_start(wq_s[:], wqT_r[:].rearrange("k p m -> p k m"))
            nc.sync.dma_start(wk_s[:], wkT_r[:].rearrange("k p m -> p k m"))
            nc.sync.dma_start(wv_s[:], wvT_r[:].rearrange("k p m -> p k m"))
            nc.sync.dma_start(wo_s[:], woT_r[:].rearrange("k p m -> p k m"))
            nc.sync.dma_start(bq_s[:], bqT[:])
            nc.sync.dma_start(bk_s[:], bkT[:])
            make_identity(nc, ident[:])

            # ---------------- Phase A: projections ----------------
            # 8 psum grid slots (1 bank each), shared via tags g0..g7 between
            # the Q/K (128,512) grids and the V (128,2,256) grid.
            with tc.tile_pool(name="xchunk", bufs=3) as xpool, \
                 tc.tile_pool(name="xvpool", bufs=1) as xvpool, \
                 tc.tile_pool(name="pjps", bufs=1, space="PSUM") as pjps:

                # QT / KT: out (128 = pair of 64-dim heads, pair j, S)
                for (xr, wsb, bsb, outsb) in (
                    (xqT_r, wq_s, bq_s, qt_s),
                    (xkT_r, wk_s, bk_s, kt_s),
                ):
                    pj = [[pjps.tile([128, 512], F32, name=f"pj{j}{n}",
                                     tag=f"g{j * NBLK + n}")
                           for n in range(NBLK)] for j in range(PAIRS)]
                    for k in range(KC):
                        xc = xpool.tile([128, S], F32R, tag="xc")
                        nc.sync.dma_start(xc[:], xr[k])
                        for j in range(PAIRS):
                            for n in range(NBLK):
                                nc.tensor.matmul(
                                    pj[j][n][:],
                                    wsb[:, k, j * 128:(j + 1) * 128],
                                    xc[:, n * 512:(n + 1) * 512],
                                    start=(k == 0), stop=(k == KC - 1))
                    for j in range(PAIRS):
                        for n in range(NBLK):
                            nc.vector.tensor_scalar_add(
                                outsb[:, j, n * 512:(n + 1) * 512],
                                pj[j][n][:], bsb[:, j:j + 1])

                # V: out (128 = sk chunk, skc, 4*64 head dims). One PSUM bank
                # per accumulation group (two groups sharing a bank corrupts
                # has_written: start=True clears bits bank-wide).
                xvc = [xvpool.tile([128, S], F32R, name=f"xv{k}", tag=f"xv{k}")
                       for k in range(KC)]
                for k in range(KC):
                    nc.sync.dma_start(xvc[k][:], xvT_r[k])
                for skc in range(SKC):
                    pvt = pjps.tile([128, HPC * DH], F32, name=f"pvt{skc}",
                                    tag=f"g{skc % 8}")
                    for k in range(KC):
                        nc.tensor.matmul(
                            pvt[:],
                            xvc[k][:, skc * 128:(skc + 1) * 128],
                            wv_s[:, k, :],
                            start=(k == 0), stop=(k == KC - 1))
                    nc.any.tensor_copy(out=v_s[:, skc, :], in_=pvt[:])

            # ---------------- Phase B: attention ----------------
            with tc.tile_pool(name="keepp", bufs=3) as keepp, \
                 tc.tile_pool(name="work", bufs=2) as work, \
                 tc.tile_pool(name="ptp", bufs=1) as ptp, \
                 tc.tile_pool(name="smallp", bufs=4) as smallp, \
                 tc.tile_pool(name="lps", bufs=2, space="PSUM") as lps, \
                 tc.tile_pool(name="tps", bufs=3, space="PSUM") as tps, \
                 tc.tile_pool(name="avps", bufs=1, space="PSUM") as avps:

                for ib in range(IBLK):
                    keeps = [keepp.tile([128, S], F32, name=f"keep{ib}_{_}", tag="keep")
                             for _ in range(2)]
                    for i2 in range(2):
                        nc.sync.dma_start(keeps[i2][:], keep_r[ib * 2 + i2])
                    for h in range(HPC):
                        j, sub = divmod(h, 2)
                        pt = ptp.tile([128, SKC, 256], F32R, tag="pt")
                        for i2 in range(2):
                            i = ib * 2 + i2
                            e_s = work.tile([128, S], F32, tag="e")
                            # QK^T logits in LG-wide psum chunks
                            for lg in range(S // LG):
                                pl = lps.tile([128, LG], F32, tag="pl")
                                for n in range(LG // 512):
                                    nc.tensor.matmul(
                                        pl[:, n * 512:(n + 1) * 512],
                                        qt_s[sub * 64:(sub + 1) * 64, j,
                                             i * 128:(i + 1) * 128],
                                        kt_s[sub * 64:(sub + 1) * 64, j,
                                             lg * LG + n * 512:
                                             lg * LG + (n + 1) * 512],
                                        start=True, stop=True)
                                nc.scalar.activation(
                                    e_s[:, lg * LG:(lg + 1) * LG], pl[:],
                                    AF.Exp, scale=0.125)
                            # mask + row sums fused
                            sums = smallp.tile([128, 1], F32, tag="sums")
                            nc.vector.scalar_tensor_tensor(
                                out=e_s[:], in0=e_s[:], scalar=1.0,
                                in1=keeps[i2][:], op0=ALU.mult, op1=ALU.mult,
                                accum_out=sums[:])
                            recip = smallp.tile([128, 1], F32, tag="recip")
                            nc.vector.reciprocal(recip[:], sums[:])
                            p_s = work.tile([128, S], F32, tag="p")
                            nc.vector.tensor_scalar_mul(
                                p_s[:], e_s[:], recip[:])
                            nc.sync.dma_start(
                                wout[h, i * 128:(i + 1) * 128, :], p_s[:])
                            # transpose P chunk-wise into PT (sk, sq) layout
                            for tg in range(SKC // 4):
                                pst = tps.tile([128, 512], F32, tag="pst")
                                for q in range(4):
                                    skc = tg * 4 + q
                                    nc.tensor.transpose(
                                        pst[:, q * 128:(q + 1) * 128],
                                        p_s[:, skc * 128:(skc + 1) * 128],
                                        ident[:])
                                nc.any.tensor_copy(
                                    out=pt[:, tg * 4:(tg + 1) * 4,
                                           i2 * 128:(i2 + 1) * 128],
                                    in_=pst[:].rearrange(
                                        "p (a b) -> p a b", a=4))
                        # PV: attnT_h block (64, 256)
                        pav = avps.tile([64, 256], F32, tag="pav")
                        for skc in range(SKC):
                            nc.tensor.matmul(
                                pav[:], v_s[:, skc, h * 64:(h + 1) * 64],
                                pt[:, skc, :],
                                start=(skc == 0), stop=(skc == SKC - 1))
                        nc.any.tensor_copy(
                            out=at_s[sub * 64:(sub + 1) * 64, j,
                                     ib * 256:(ib + 1) * 256],
                            in_=pav[:])

            # ---------------- Phase C: output projection ----------------
            with tc.tile_pool(name="op", bufs=2) as op, \
                 tc.tile_pool(name="ops", bufs=4, space="PSUM") as ops:
                for i in range(SQC):
                    po_s = op.tile([128, D], F32, tag="po")
                    for n in range(D // 512):
                        pso = ops.tile([128, 512], F32, tag="pso")
                        for kc in range(PAIRS):
                            nc.tensor.matmul(
                                pso[:],
                                at_s[:, kc, i * 128:(i + 1) * 128],
                                wo_s[:, kc, n * 512:(n + 1) * 512],
                                start=(kc == 0), stop=(kc == PAIRS - 1))
                        nc.any.tensor_copy(
                            out=po_s[:, n * 512:(n + 1) * 512], in_=pso[:])
                    nc.sync.dma_start(pout[i * 128:(i + 1) * 128, :], po_s[:])

    nc.compile()
    return nc


_NC_CACHE = {}


def _get_nc():
    if "nc" not in _NC_CACHE:
        _NC_CACHE["nc"] = build_kernel(NCORES)
    return _NC_CACHE["nc"]


def kernel(queries, keys, values, mask, Wq, bq, Wk, bk, Wv, bv, Wo, bo,
           _trace=False):
    queries = np.asarray(queries, dtype=np.float32)
    keys = np.asarray(keys, dtype=np.float32)
    values = np.asarray(values, dtype=np.float32)
    mask = np.asarray(mask)
    Wq = np.asarray(Wq, dtype=np.float32)
    Wk = np.asarray(Wk, dtype=np.float32)
    Wv = np.asarray(Wv, dtype=np.float32)
    Wo = np.asarray(Wo, dtype=np.float32)
    bq = np.asarray(bq, dtype=np.float32)
    bk = np.asarray(bk, dtype=np.float32)
    bv = np.asarray(bv, dtype=np.float32)
    bo = np.asarray(bo, dtype=np.float32)

    nc = _get_nc()

    xT = {}
    keeps = {}
    for b in range(B):
        xT[b] = (np.ascontiguousarray(queries[b].T),
                 np.ascontiguousarray(keys[b].T),
                 np.ascontiguousarray(values[b].T))
        keeps[b] = np.ascontiguousarray(
            (1 - mask[b, 0]).astype(np.float32))

    in_maps = []
    for c in range(NCORES):
        b = c // 4
        h0 = HPC * (c % 4)
        sl = slice(h0 * DH, (h0 + HPC) * DH)
        xq, xk, xv = xT[b]
        in_maps.append({
            "xqT": xq, "xkT": xk, "xvT": xv, "keep": keeps[b],
            "wqT": np.ascontiguousarray(Wq[sl, :].T),
            "wkT": np.ascontiguousarray(Wk[sl, :].T),
            "wvT": np.ascontiguousarray(Wv[sl, :].T),
            "woT": np.ascontiguousarray(Wo[:, sl].T),
            "bqT": np.ascontiguousarray(bq[sl].reshape(PAIRS, 128).T),
            "bkT": np.ascontiguousarray(bk[sl].reshape(PAIRS, 128).T),
        })

    res = run_bass_kernel_spmd(nc, in_maps, core_ids=list(range(NCORES)),
                               trace=_trace)

    attention = np.empty((B, S, D), dtype=np.float32)
    weights = np.empty((B, H, S, S), dtype=np.float32)
    # bv folds through: P @ (V + 1 bv^T) = P@V + 1 bv^T; then Wo^T and bo.
    host_bias = bv @ Wo.T + bo
    for b in range(B):
        acc = None
        for cc in range(4):
            c = b * 4 + cc
            r = res.results[c]
            weights[b, HPC * cc:HPC * (cc + 1)] = r["wout"]
            acc = r["pout"] if acc is None else acc + r["pout"]
        attention[b] = acc + host_bias
    kernel.last_results = res
    if _trace:
        kernel.last_exec_time_ns = res.exec_time_ns
    return attention, weights


if __name__ == "__main__":
    rng = np.random.default_rng(0)
    inputs = {
        "queries": rng.standard_normal((B, S, D), dtype=np.float32),
        "keys": rng.standard_normal((B, S, D), dtype=np.float32),
        "values": rng.standard_normal((B, S, D), dtype=np.float32),
        "mask": rng.integers(0, 2, (B, 1, S, S)).astype(np.int32),
        "Wq": rng.standard_normal((D, D), dtype=np.float32) * 0.02,
        "bq": np.zeros(D, np.float32),
        "Wk": rng.standard_normal((D, D), dtype=np.float32) * 0.02,
        "bk": np.zeros(D, np.float32),
        "Wv": rng.standard_normal((D, D), dtype=np.float32) * 0.02,
        "bv": np.zeros(D, np.float32),
        "Wo": rng.standard_normal((D, D), dtype=np.float32) * 0.02,
        "bo": np.zeros(D, np.float32),
    }
    out, w = kernel(**inputs)
    print("ran", out.shape, w.shape)
